# revision 12
# baseline (speedup 1.0000x reference)
"""3x3 valid conv (cross-correlation) + bias on a 4096x4096 fp32 image,
run across 8 trn2 NeuronCores.

Strategy
--------
Rows are sharded across the 8 cores host-side with a 2-row halo folded
into each core's input slice (no device collectives). On each core the
conv is computed as banded matmuls on the TensorEngine:

  For an output row-tile of M=126 rows (input rows K=128), and each of
  the 3 kernel columns dj, a banded stationary matrix
  B_dj[k, m] = w[k-m, dj] (zero outside 0<=k-m<=2) gives

      Y_tile[m, n] = sum_dj sum_k B_dj[k, m] * X_tile[k, n+dj]

  as 3 accumulating matmuls per 1024-wide output chunk (fp16 rhs may
  stream 1024 columns; the fp32 PSUM group spans 2 banks).

Pipeline design (what the previous 52-59us version got wrong):

* The PE clock is HAM-gated: cold = 1.2GHz (really ~1.0 under P0),
  warm = 2.4 (really ~2.0). Warm needs ~3.4us of sustained activity and
  any >3.4us idle re-throttles. So: warm-up matmuls on a DVE-memset
  tile start ~0.5us into the kernel (the old version waited 7.5us on a
  slow GPSIMD memset) and the real matmul stream is scheduled to never
  stall more than a few hundred ns.
* Critical-path first loads are minimal: the full-tile band planes
  (98KB) + an fp32 bias column + the first quarter of tile 0. Tile 0
  is loaded as 4 overlapping column quarters so group g only waits for
  quarter g. Everything loads on the Sync HWDGE queue in need order.
* Stores ride the Scalar HWDGE queue (decoupled from the load queue's
  FIFO so an early store is not stuck behind the tile-3 load), with
  the PSUM->SBUF bias-add copies alternating DVE/ACT per group.
* PSUM: 3 double-bank group tiles pipeline the matmul/copy overlap
  plus 1 single-bank tile for warm-up and the remainder strip.

The remainder strip (512 = 4*126 + 8 rows) is computed as one 512-wide
matmul set over NSEG=8 host-packed column segments stacked across
partitions, scheduled right after tile 0 so its copy/store stay off
the critical tail.
"""

import numpy as np

H = 4096
W = 4096
KH = 3
KW = 3
HOUT = H - KH + 1  # 4094
WOUT = W - KW + 1  # 4094
NCORES = 8
ROWS_PER_CORE = 512          # output rows computed per core
IN_ROWS = ROWS_PER_CORE + 2  # input rows per core (with halo)
# Core 7 overlaps core 6 by 2 rows so that all shards have equal shape.
STARTS = [0, 512, 1024, 1536, 2048, 2560, 3072, 3582]
M_TILE = 126                 # output rows per matmul tile (K = M + 2 = 128)
N_FULL_TILES = ROWS_PER_CORE // M_TILE   # 4
GRP_W = 1024                 # output columns per PSUM group (2 banks fp32)
QUART_W = GRP_W + 2          # tile-0 quarter width (dj halo)

# remainder strip: last R_STRIP output rows, packed as NSEG column
# segments stacked across partitions (NSEG*STRIP_IN partitions)
R_STRIP = ROWS_PER_CORE - N_FULL_TILES * M_TILE  # 8
STRIP_IN = R_STRIP + 2                   # 10
NSEG = 8
SEG = W // NSEG                          # 512

N_WARM = 9                   # PE p-state warm-up matmuls (512 cols each)
VARIANT = "f16o"

_PROGRAM_CACHE = {}


def _build_program(variant: str, bias_val: float):
    import concourse.mybir as mybir
    from concourse import bacc
    from concourse.tile import TileContext

    assert variant == "f16o", variant
    f32 = mybir.dt.float32
    f16 = mybir.dt.float16
    out_dt = f16

    nc = bacc.Bacc()
    x = nc.declare_dram_parameter("x0", [IN_ROWS, W], f16, isOutput=False)
    bandT = nc.declare_dram_parameter("bandT", [128, KW * 128], f16, isOutput=False)
    bandS = nc.declare_dram_parameter("bandS", [128, KW * 128], f16, isOutput=False)
    s0 = nc.declare_dram_parameter(
        "s0", [NSEG * STRIP_IN, SEG + 2], f16, isOutput=False
    )
    y = nc.declare_dram_parameter("y", [ROWS_PER_CORE, WOUT], out_dt, isOutput=True)
    ysd = nc.declare_dram_parameter("ys", [NSEG * R_STRIP, SEG], out_dt, isOutput=True)

    # output column groups
    groups = []
    c0 = 0
    while c0 < WOUT:
        groups.append((c0, min(GRP_W, WOUT - c0)))
        c0 += GRP_W
    NG = len(groups)  # 4

    npart = NSEG * STRIP_IN           # 80
    nout = NSEG * R_STRIP             # 64

    with TileContext(nc) as tc:
        with (
            tc.tile_pool(name="consts", bufs=1) as consts,
            tc.tile_pool(name="xq", bufs=NG) as xqp,
            tc.tile_pool(name="xp", bufs=N_FULL_TILES - 1) as xp,
            tc.tile_pool(name="sp", bufs=1) as sp,
            tc.tile_pool(name="yv", bufs=8) as yvp,
            tc.tile_pool(name="ysc", bufs=8) as yscp,
            tc.tile_pool(name="yst", bufs=1) as ystp,
            tc.tile_pool(name="pp", bufs=4, space="PSUM") as pp,
        ):
            # PE p-state warm-up tile: DVE memset (fast, no DMA dep) so
            # dummy matmuls start within ~0.5us and hold the HAM window
            # open until real data lands.
            warm = consts.tile([128, 512], f16)
            nc.vector.memset(warm[:, :], 0)

            # loads, Sync HWDGE queue, in need order
            bandT_sb = consts.tile([128, KW * 128], f16)
            nc.sync.dma_start(out=bandT_sb[:], in_=bandT[:])
            k0 = M_TILE + KH - 1  # 128
            xqs = []
            for g, (c0, w) in enumerate(groups):
                qw = min(QUART_W, W - c0)
                t = xqp.tile([128, QUART_W], f16, tag="xq")
                nc.sync.dma_start(out=t[:k0, :qw], in_=x[0:k0, c0 : c0 + qw])
                xqs.append(t)
            xts = [None]
            for t_i in range(1, N_FULL_TILES):
                r0 = t_i * M_TILE
                xt = xp.tile([128, W], f16, tag="x")
                nc.sync.dma_start(out=xt[:k0, :], in_=x[r0 : r0 + k0, :])
                xts.append(xt)
            bandS_sb = consts.tile([128, KW * 128], f16)
            nc.sync.dma_start(out=bandS_sb[:], in_=bandS[:])
            strip = sp.tile([npart, SEG + 2], f16)
            nc.sync.dma_start(out=strip[:, :], in_=s0[:, :])

            bandT_r = bandT_sb.rearrange("p (a b) -> p a b", b=128)
            bandS_r = bandS_sb.rearrange("p (a b) -> p a b", b=128)

            # warm-up matmuls into the first ring slot (no copy ever
            # reads it; group 3 recycles the slot once warm-up mms retire)
            pw = pp.tile([128, GRP_W], f32, tag="pt")
            for _ in range(N_WARM):
                nc.tensor.matmul(
                    pw[:128, :512], warm[:, :128], warm[:, :512],
                    start=True, stop=True,
                )

            gidx = 0  # global group counter for DVE/ACT alternation

            def do_group(t_i, g):
                nonlocal gidx
                r0 = t_i * M_TILE
                m = M_TILE
                c0, w = groups[g]
                pt = pp.tile([128, GRP_W], f32, tag="pt")
                # matmul out is capped at 512 fp32 elements -> 2 chunks per
                # group, weight-major so each LDWEIGHTS serves both chunks
                chunks = [(o, min(512, w - o)) for o in (0, 512) if o < w]
                for dj in range(KW):
                    lhsT = bandT_r[:k0, dj, :]
                    for o, cw in chunks:
                        if t_i == 0:
                            rhs = xqs[g][:k0, o + dj : o + dj + cw]
                        else:
                            rhs = xts[t_i][:k0, c0 + o + dj : c0 + o + dj + cw]
                        nc.tensor.matmul(
                            pt[:128, o : o + cw], lhsT, rhs,
                            start=(dj == 0), stop=(dj == KW - 1),
                        )
                # bias as an instruction immediate — a [128,1] bias DMA's
                # completion sem lands ~6us late on the busy ring and was
                # gating both copy engines
                if gidx % 2 == 0:
                    yt = yvp.tile([128, GRP_W], out_dt, tag="yv")
                    nc.vector.tensor_scalar_add(
                        yt[:m, :w], pt[:m, :w], bias_val
                    )
                else:
                    yt = yscp.tile([128, GRP_W], out_dt, tag="ysc")
                    nc.scalar.activation(
                        yt[:m, :w], pt[:m, :w],
                        mybir.ActivationFunctionType.Copy,
                        bias=bias_val,
                    )
                nc.sync.dma_start(
                    out=y[r0 : r0 + m, c0 : c0 + w], in_=yt[:m, :w]
                )
                gidx += 1

            for t_i in range(N_FULL_TILES):
                for g in range(NG):
                    do_group(t_i, g)

            # remainder strip last: its small copy/store makes the shortest
            # possible drain tail
            ptS = pp.tile([128, GRP_W], f32, tag="pt")
            for dj in range(KW):
                nc.tensor.matmul(
                    ptS[:128, :SEG],
                    bandS_r[:npart, dj, :],
                    strip[:npart, dj : dj + SEG],
                    start=(dj == 0), stop=(dj == KW - 1),
                )
            ys = ystp.tile([nout, SEG], out_dt, tag="ystrip")
            nc.vector.tensor_scalar_add(
                ys[:, :], ptS[:nout, :SEG], bias_val
            )
            nc.sync.dma_start(out=ysd[:, :], in_=ys[:, :])
    nc.finalize()
    return nc


def _get_program(variant: str, bias_val: float):
    key = (variant, bias_val)
    if key not in _PROGRAM_CACHE:
        _PROGRAM_CACHE[key] = _build_program(variant, bias_val)
    return _PROGRAM_CACHE[key]


def _make_bands(w):
    """w: [KH, KW] fp16. Returns (bandT, bandS), each [128, KW*128].

    bandT planes: full 128-wide bands (columns >= M_TILE produce garbage
    rows that never leave PSUM, but make NumWeights==128 for FWL).
    bandS planes: block-diagonal strip bands."""
    dtype = w.dtype
    bt = np.zeros((128, KW, 128), dtype)
    bs = np.zeros((128, KW, 128), dtype)
    for dj in range(KW):
        for d in range(KH):
            idx = np.arange(128 - d)
            bt[idx + d, dj, idx] = w[d, dj]
        for blk in range(NSEG):
            for rp in range(R_STRIP):
                for d in range(KH):
                    bs[
                        STRIP_IN * blk + rp + d,
                        dj,
                        R_STRIP * blk + rp,
                    ] = w[d, dj]
    return bt.reshape(128, -1), bs.reshape(128, -1)


def _run(X, weight, bias, trace=False, variant=None):
    from concourse.bass_utils import run_bass_kernel_spmd

    variant = variant or VARIANT
    X = np.ascontiguousarray(np.asarray(X, dtype=np.float32))
    w = np.asarray(weight, dtype=np.float32)
    b = np.asarray(bias, dtype=np.float32)
    assert X.shape == (H, W) and w.shape == (KH, KW)

    nc = _get_program(variant, float(b[0]))

    Xh = X.astype(np.float16)
    bandT, bandS = _make_bands(w.astype(np.float16))

    def pack_strip(xp_arr, s):
        rs = s + N_FULL_TILES * M_TILE
        strip = xp_arr[rs : rs + STRIP_IN]  # [10, 4096]
        packed = np.zeros((NSEG * STRIP_IN, SEG + 2), xp_arr.dtype)
        packed[:, :SEG] = (
            strip.reshape(STRIP_IN, NSEG, SEG).transpose(1, 0, 2).reshape(-1, SEG)
        )
        halo = (
            strip[:, SEG:]
            .reshape(STRIP_IN, NSEG - 1, SEG)
            .transpose(1, 0, 2)
            .reshape(-1, SEG)[:, :2]
        )
        packed[: (NSEG - 1) * STRIP_IN, SEG : SEG + 2] = halo
        return packed

    in_maps = []
    for s in STARTS:
        in_maps.append(
            {
                "x0": Xh[s : s + IN_ROWS],
                "bandT": bandT,
                "bandS": bandS,
                "s0": pack_strip(Xh, s),
            }
        )
    res = run_bass_kernel_spmd(
        nc, in_maps, core_ids=list(range(NCORES)), trace=trace
    )

    def core_block(c, blk):
        r = res.results[c]
        blk[: N_FULL_TILES * M_TILE] = r["y"][: N_FULL_TILES * M_TILE]
        ys = r["ys"]  # [NSEG*R_STRIP, SEG] packed strip output
        for b_ in range(NSEG):
            wdt = min(SEG, WOUT - b_ * SEG)
            blk[N_FULL_TILES * M_TILE :, b_ * SEG : b_ * SEG + wdt] = ys[
                b_ * R_STRIP : (b_ + 1) * R_STRIP, :wdt
            ]

    out = np.empty((HOUT, WOUT), np.float32)
    for c in range(NCORES - 1):
        core_block(c, out[STARTS[c] : STARTS[c] + ROWS_PER_CORE])
    last = np.empty((ROWS_PER_CORE, WOUT), np.float32)
    core_block(NCORES - 1, last)
    out[STARTS[-1] + 2 :] = last[2:]
    return out, (res if trace else None)


def kernel(X, weight, bias):
    out, _ = _run(X, weight, bias, trace=False)
    return out


# revision 13
# speedup vs baseline: 1.0063x; 1.0063x over previous
"""3x3 valid conv (cross-correlation) + bias on a 4096x4096 fp32 image,
run across 8 trn2 NeuronCores.

Strategy
--------
Rows are sharded across the 8 cores host-side with a 2-row halo folded
into each core's input slice (no device collectives). On each core the
conv is computed as banded matmuls on the TensorEngine:

  For an output row-tile of M=126 rows (input rows K=128), and each of
  the 3 kernel columns dj, a banded stationary matrix
  B_dj[k, m] = w[k-m, dj] (zero outside 0<=k-m<=2) gives

      Y_tile[m, n] = sum_dj sum_k B_dj[k, m] * X_tile[k, n+dj]

  as 3 accumulating matmuls per 1024-wide output chunk (fp16 rhs may
  stream 1024 columns; the fp32 PSUM group spans 2 banks).

Pipeline design (what the previous 52-59us version got wrong):

* The PE clock is HAM-gated: cold = 1.2GHz (really ~1.0 under P0),
  warm = 2.4 (really ~2.0). Warm needs ~3.4us of sustained activity and
  any >3.4us idle re-throttles. So: warm-up matmuls on a DVE-memset
  tile start ~0.5us into the kernel (the old version waited 7.5us on a
  slow GPSIMD memset) and the real matmul stream is scheduled to never
  stall more than a few hundred ns.
* Critical-path first loads are minimal: the full-tile band planes
  (98KB) + an fp32 bias column + the first quarter of tile 0. Tile 0
  is loaded as 4 overlapping column quarters so group g only waits for
  quarter g. Everything loads on the Sync HWDGE queue in need order.
* Stores ride the Scalar HWDGE queue (decoupled from the load queue's
  FIFO so an early store is not stuck behind the tile-3 load), with
  the PSUM->SBUF bias-add copies alternating DVE/ACT per group.
* PSUM: 3 double-bank group tiles pipeline the matmul/copy overlap
  plus 1 single-bank tile for warm-up and the remainder strip.

The remainder strip (512 = 4*126 + 8 rows) is computed as one 512-wide
matmul set over NSEG=8 host-packed column segments stacked across
partitions, scheduled right after tile 0 so its copy/store stay off
the critical tail.
"""

import numpy as np

H = 4096
W = 4096
KH = 3
KW = 3
HOUT = H - KH + 1  # 4094
WOUT = W - KW + 1  # 4094
NCORES = 8
ROWS_PER_CORE = 512          # output rows computed per core
IN_ROWS = ROWS_PER_CORE + 2  # input rows per core (with halo)
# Core 7 overlaps core 6 by 2 rows so that all shards have equal shape.
STARTS = [0, 512, 1024, 1536, 2048, 2560, 3072, 3582]
M_TILE = 126                 # output rows per matmul tile (K = M + 2 = 128)
N_FULL_TILES = ROWS_PER_CORE // M_TILE   # 4
GRP_W = 1024                 # output columns per PSUM group (2 banks fp32)
QUART_W = GRP_W + 2          # tile-0 quarter width (dj halo)

# remainder strip: last R_STRIP output rows, packed as NSEG column
# segments stacked across partitions (NSEG*STRIP_IN partitions)
R_STRIP = ROWS_PER_CORE - N_FULL_TILES * M_TILE  # 8
STRIP_IN = R_STRIP + 2                   # 10
NSEG = 8
SEG = W // NSEG                          # 512

N_WARM = 9                   # PE p-state warm-up matmuls (512 cols each)
VARIANT = "f16o"

_PROGRAM_CACHE = {}


def _build_program(variant: str, bias_val: float):
    import concourse.mybir as mybir
    from concourse import bacc
    from concourse.tile import TileContext

    assert variant == "f16o", variant
    f32 = mybir.dt.float32
    f16 = mybir.dt.float16
    out_dt = f16

    nc = bacc.Bacc()
    x = nc.declare_dram_parameter("x0", [IN_ROWS, W], f16, isOutput=False)
    bandT = nc.declare_dram_parameter("bandT", [128, KW * 128], f16, isOutput=False)
    bandS = nc.declare_dram_parameter("bandS", [128, KW * 128], f16, isOutput=False)
    s0 = nc.declare_dram_parameter(
        "s0", [NSEG * STRIP_IN, SEG + 2], f16, isOutput=False
    )
    y = nc.declare_dram_parameter("y", [ROWS_PER_CORE, WOUT], out_dt, isOutput=True)
    ysd = nc.declare_dram_parameter("ys", [NSEG * R_STRIP, SEG], out_dt, isOutput=True)

    # output column groups
    groups = []
    c0 = 0
    while c0 < WOUT:
        groups.append((c0, min(GRP_W, WOUT - c0)))
        c0 += GRP_W
    NG = len(groups)  # 4

    npart = NSEG * STRIP_IN           # 80
    nout = NSEG * R_STRIP             # 64

    with TileContext(nc) as tc:
        with (
            tc.tile_pool(name="consts", bufs=1) as consts,
            tc.tile_pool(name="xq", bufs=NG) as xqp,
            tc.tile_pool(name="xp", bufs=N_FULL_TILES - 1) as xp,  # per-tag
            tc.tile_pool(name="sp", bufs=1) as sp,
            tc.tile_pool(name="yv", bufs=8) as yvp,
            tc.tile_pool(name="ysc", bufs=8) as yscp,
            tc.tile_pool(name="yst", bufs=1) as ystp,
            tc.tile_pool(name="pp", bufs=4, space="PSUM") as pp,
        ):
            # PE p-state warm-up tile: DVE memset (fast, no DMA dep) so
            # dummy matmuls start within ~0.5us and hold the HAM window
            # open until real data lands.
            warm = consts.tile([128, 512], f16)
            nc.vector.memset(warm[:, :], 0)

            # loads, Sync HWDGE queue, in need order
            bandT_sb = consts.tile([128, KW * 128], f16)
            nc.sync.dma_start(out=bandT_sb[:], in_=bandT[:])
            k0 = M_TILE + KH - 1  # 128
            # quarter 0 splits into two 512-col sub-chunks so the very first
            # matmul only waits for ~0.4us of image data
            xq0a = xqp.tile([128, 516], f16, tag="xq0a")
            nc.sync.dma_start(out=xq0a[:k0, :], in_=x[0:k0, 0:516])
            xq0b = xqp.tile([128, 514], f16, tag="xq0b")
            nc.sync.dma_start(out=xq0b[:k0, :], in_=x[0:k0, 512:1026])
            xqs = [None]
            for g, (c0, w) in list(enumerate(groups))[1:]:
                qw = min(QUART_W, W - c0)
                t = xqp.tile([128, QUART_W], f16, tag="xq")
                nc.sync.dma_start(out=t[:k0, :qw], in_=x[0:k0, c0 : c0 + qw])
                xqs.append(t)
            # tiles 1..3 load as overlapping column halves: groups 0-1
            # only wait for half A, so the PE never outruns the load stream
            HALF_W = 2 * GRP_W + 2  # 2050
            xts = [None]
            for t_i in range(1, N_FULL_TILES):
                r0 = t_i * M_TILE
                ha = xp.tile([128, HALF_W], f16, tag="xha")
                nc.sync.dma_start(out=ha[:k0, :], in_=x[r0 : r0 + k0, 0:HALF_W])
                hb = xp.tile([128, W - 2 * GRP_W], f16, tag="xhb")
                nc.sync.dma_start(
                    out=hb[:k0, :], in_=x[r0 : r0 + k0, 2 * GRP_W : W]
                )
                xts.append((ha, hb))
            bandS_sb = consts.tile([128, KW * 128], f16)
            nc.sync.dma_start(out=bandS_sb[:], in_=bandS[:])
            strip = sp.tile([npart, SEG + 2], f16)
            nc.sync.dma_start(out=strip[:, :], in_=s0[:, :])

            bandT_r = bandT_sb.rearrange("p (a b) -> p a b", b=128)
            bandS_r = bandS_sb.rearrange("p (a b) -> p a b", b=128)

            # warm-up matmuls into the first ring slot (no copy ever
            # reads it; group 3 recycles the slot once warm-up mms retire)
            pw = pp.tile([128, GRP_W], f32, tag="pt")
            for _ in range(N_WARM):
                nc.tensor.matmul(
                    pw[:128, :512], warm[:, :128], warm[:, :512],
                    start=True, stop=True,
                )

            gidx = 0  # global group counter for DVE/ACT alternation

            def do_group(t_i, g):
                nonlocal gidx
                r0 = t_i * M_TILE
                m = M_TILE
                c0, w = groups[g]
                pt = pp.tile([128, GRP_W], f32, tag="pt")
                # matmul out is capped at 512 fp32 elements -> 2 chunks per
                # group, weight-major so each LDWEIGHTS serves both chunks
                chunks = [(o, min(512, w - o)) for o in (0, 512) if o < w]
                for dj in range(KW):
                    lhsT = bandT_r[:k0, dj, :]
                    for o, cw in chunks:
                        if t_i == 0:
                            if g == 0:
                                src_t = xq0a if o == 0 else xq0b
                                rhs = src_t[:k0, dj : dj + cw]
                            else:
                                rhs = xqs[g][:k0, o + dj : o + dj + cw]
                        elif g < 2:
                            rhs = xts[t_i][0][:k0, c0 + o + dj : c0 + o + dj + cw]
                        else:
                            base = 2 * GRP_W
                            rhs = xts[t_i][1][
                                :k0, c0 + o + dj - base : c0 + o + dj - base + cw
                            ]
                        nc.tensor.matmul(
                            pt[:128, o : o + cw], lhsT, rhs,
                            start=(dj == 0), stop=(dj == KW - 1),
                        )
                # bias as an instruction immediate — a [128,1] bias DMA's
                # completion sem lands ~6us late on the busy ring and was
                # gating both copy engines
                if gidx % 2 == 0:
                    yt = yvp.tile([128, GRP_W], out_dt, tag="yv")
                    nc.vector.tensor_scalar_add(
                        yt[:m, :w], pt[:m, :w], bias_val
                    )
                else:
                    yt = yscp.tile([128, GRP_W], out_dt, tag="ysc")
                    nc.scalar.activation(
                        yt[:m, :w], pt[:m, :w],
                        mybir.ActivationFunctionType.Copy,
                        bias=bias_val,
                    )
                nc.sync.dma_start(
                    out=y[r0 : r0 + m, c0 : c0 + w], in_=yt[:m, :w]
                )
                gidx += 1

            for t_i in range(N_FULL_TILES):
                for g in range(NG):
                    do_group(t_i, g)

            # remainder strip last: its small copy/store makes the shortest
            # possible drain tail
            ptS = pp.tile([128, GRP_W], f32, tag="pt")
            for dj in range(KW):
                nc.tensor.matmul(
                    ptS[:128, :SEG],
                    bandS_r[:npart, dj, :],
                    strip[:npart, dj : dj + SEG],
                    start=(dj == 0), stop=(dj == KW - 1),
                )
            ys = ystp.tile([nout, SEG], out_dt, tag="ystrip")
            nc.vector.tensor_scalar_add(
                ys[:, :], ptS[:nout, :SEG], bias_val
            )
            nc.sync.dma_start(out=ysd[:, :], in_=ys[:, :])
    nc.finalize()
    return nc


def _get_program(variant: str, bias_val: float):
    key = (variant, bias_val)
    if key not in _PROGRAM_CACHE:
        _PROGRAM_CACHE[key] = _build_program(variant, bias_val)
    return _PROGRAM_CACHE[key]


def _make_bands(w):
    """w: [KH, KW] fp16. Returns (bandT, bandS), each [128, KW*128].

    bandT planes: full 128-wide bands (columns >= M_TILE produce garbage
    rows that never leave PSUM, but make NumWeights==128 for FWL).
    bandS planes: block-diagonal strip bands."""
    dtype = w.dtype
    bt = np.zeros((128, KW, 128), dtype)
    bs = np.zeros((128, KW, 128), dtype)
    for dj in range(KW):
        for d in range(KH):
            idx = np.arange(128 - d)
            bt[idx + d, dj, idx] = w[d, dj]
        for blk in range(NSEG):
            for rp in range(R_STRIP):
                for d in range(KH):
                    bs[
                        STRIP_IN * blk + rp + d,
                        dj,
                        R_STRIP * blk + rp,
                    ] = w[d, dj]
    return bt.reshape(128, -1), bs.reshape(128, -1)


def _run(X, weight, bias, trace=False, variant=None):
    from concourse.bass_utils import run_bass_kernel_spmd

    variant = variant or VARIANT
    X = np.ascontiguousarray(np.asarray(X, dtype=np.float32))
    w = np.asarray(weight, dtype=np.float32)
    b = np.asarray(bias, dtype=np.float32)
    assert X.shape == (H, W) and w.shape == (KH, KW)

    nc = _get_program(variant, float(b[0]))

    Xh = X.astype(np.float16)
    bandT, bandS = _make_bands(w.astype(np.float16))

    def pack_strip(xp_arr, s):
        rs = s + N_FULL_TILES * M_TILE
        strip = xp_arr[rs : rs + STRIP_IN]  # [10, 4096]
        packed = np.zeros((NSEG * STRIP_IN, SEG + 2), xp_arr.dtype)
        packed[:, :SEG] = (
            strip.reshape(STRIP_IN, NSEG, SEG).transpose(1, 0, 2).reshape(-1, SEG)
        )
        halo = (
            strip[:, SEG:]
            .reshape(STRIP_IN, NSEG - 1, SEG)
            .transpose(1, 0, 2)
            .reshape(-1, SEG)[:, :2]
        )
        packed[: (NSEG - 1) * STRIP_IN, SEG : SEG + 2] = halo
        return packed

    in_maps = []
    for s in STARTS:
        in_maps.append(
            {
                "x0": Xh[s : s + IN_ROWS],
                "bandT": bandT,
                "bandS": bandS,
                "s0": pack_strip(Xh, s),
            }
        )
    res = run_bass_kernel_spmd(
        nc, in_maps, core_ids=list(range(NCORES)), trace=trace
    )

    def core_block(c, blk):
        r = res.results[c]
        blk[: N_FULL_TILES * M_TILE] = r["y"][: N_FULL_TILES * M_TILE]
        ys = r["ys"]  # [NSEG*R_STRIP, SEG] packed strip output
        for b_ in range(NSEG):
            wdt = min(SEG, WOUT - b_ * SEG)
            blk[N_FULL_TILES * M_TILE :, b_ * SEG : b_ * SEG + wdt] = ys[
                b_ * R_STRIP : (b_ + 1) * R_STRIP, :wdt
            ]

    out = np.empty((HOUT, WOUT), np.float32)
    for c in range(NCORES - 1):
        core_block(c, out[STARTS[c] : STARTS[c] + ROWS_PER_CORE])
    last = np.empty((ROWS_PER_CORE, WOUT), np.float32)
    core_block(NCORES - 1, last)
    out[STARTS[-1] + 2 :] = last[2:]
    return out, (res if trace else None)


def kernel(X, weight, bias):
    out, _ = _run(X, weight, bias, trace=False)
    return out
